# revision 5
# baseline (speedup 1.0000x reference)
"""Hopfield neuron update kernel for 8 Trainium2 NeuronCores (fp8 stream).

act = W @ s - diag(W)*s + (N-1)*b;  new_state = sign(act)  (host-side).

Memory-bound on the W stream -> minimize streamed bytes: s is +-1 so
products are exact in any W precision; only the quantization of W
matters (act scale ~7e4 vs fp8-e4m3 row error <= ~16, 2e-4 relative).
Rows where |c| = |(N-1)b - diag*s| < 600 could flip new_state's sign
under fp8 error (|W@s| <= ~520 + err), so those (~485 of 16384) stream
in bf16 on the Vector-engine path; all other rows stream as 1-byte fp8
e4m3 (TRN FP8_EXP4-compatible, |W| < 6 << 240) through the
TensorEngine's Double-FP8 matmul (DoubleRow perf mode: K=256 per
instruction, 2 fp8 per lane-cycle; the k-pair dim of both operands must
be OUTER per the s3_lw_dual_fp8 ldweights ISA rule). ~34.6 MiB/core at
the ~22.4 GB/s x 16 DMA-engine per-NC HBM limit -> ~101 us stream, the
binding roofline; everything else hides under or drains right after it:
  - W^T tiles merge two chunk-pairs (7936 B per-partition lines --
    measured the best descriptor size: 3968 B lines ran 6% slower,
    16 KiB lines slower still) and alternate between the two HWDGE
    rings (sync/scalar) so each DMA engine round-robins two queues and
    per-queue stalls overlap (measured ~-10 us vs a single ring); the
    final tile splits in two so the PE drain after the last byte is
    short;
  - the rank-1 ones-outer-product broadcast of s (PE matmul + ACT copy
    per 512-col chunk) is interleaved into the first 32 chunk-pairs so
    the in-order PE queue reaches the first fp8 matmul immediately;
  - DVE bf16 dot products run mid-stream off 4 interleaved 512 KiB
    tiles scheduled in the first half of the ring (a DVE op must be
    emitted after the broadcast copies of every sb chunk it reads --
    earlier emission is a WAR race reading uninitialized SBUF);
  - per-PSUM-block epilogue add + output DMA pipeline behind the stop
    matmuls; corrections c fold host-side; new_state = sign(act) and
    the row un-permute are O(N) host postprocess.
Measured 117.3-117.5 us on the 8-core axon pod (baseline bf16: 211-221
us); the stream runs at ~95% of device HBM bandwidth.
"""

import os
import sys

import ml_dtypes
import numpy as np

for _p in ("/opt/trn_rl_repo", "/root/.axon_site/_ro/trn_rl_repo"):
    if os.path.isdir(_p) and _p not in sys.path:
        sys.path.insert(0, _p)

N = 16384
NCORES = 8
R = N // NCORES          # rows per core: 2048
P = 128                  # SBUF partitions
RISK_T = 600.0           # |c| below this -> bf16 row (margin ~373 on seed 0)
RB = 64                  # bf16 rows per core (>= ceil(risky/8), <= 128)
RF = R - RB              # TensorE fp8 rows per core: 1984
FD = 4096                # DVE op chunk (elems)
NCH = N // FD            # DVE chunks per row: 4
NKP = N // 256           # chunk-pairs for DoubleRow accumulation: 64
BLOCKS = (512, 512, 512, RF - 1536)  # PSUM out-free blocks
TBUFS = 8                # in-flight TensorE W^T tiles
WBUFS = 4                # in-flight DVE W tiles
BCF = 512                # broadcast chunk (PSUM bank max)

_CACHE = {}


def _build_nc():
    import concourse.bacc as bacc
    import concourse.mybir as mybir
    from concourse.tile import TileContext

    f32 = mybir.dt.float32
    bf = mybir.dt.bfloat16
    f8 = mybir.dt.float8e4
    nc = bacc.Bacc()

    wte8 = nc.dram_tensor("wte8", [NKP // 2, P, 4 * RF], f8, kind="ExternalInput")
    w = nc.dram_tensor("w", [RB, N], bf, kind="ExternalInput")
    s8 = nc.dram_tensor("s8", [P, 2 * NKP], f8, kind="ExternalInput")
    srow8 = nc.dram_tensor("srow8", [N], f8, kind="ExternalInput")
    c_t = nc.dram_tensor("c_t", [RB, 1], f32, kind="ExternalInput")
    c_te = nc.dram_tensor("c_te", [1, RF], f32, kind="ExternalInput")
    out_o = nc.dram_tensor("out_o", [RB, 1], f32, kind="ExternalOutput")
    out_te = nc.dram_tensor("out_te", [1, RF], f32, kind="ExternalOutput")

    with TileContext(nc) as tc:
        with (
            tc.tile_pool(name="consts", bufs=1) as consts,
            tc.tile_pool(name="tpool", bufs=TBUFS) as tpool,
            tc.tile_pool(name="wpool", bufs=WBUFS) as wpool,
            tc.tile_pool(name="psacc", bufs=1, space="PSUM") as psacc,
            tc.tile_pool(name="bcpsum", bufs=4, space="PSUM") as bcpsum,
        ):
            sb = consts.tile([P, N], f8)
            ss = consts.tile([P, 2, NKP], f8)
            srow = consts.tile([1, N], f8)
            ones = consts.tile([1, P], f8)
            dummy = consts.tile([RB, 1], f8)
            partials = consts.tile([RB, NCH], f32)
            ct = consts.tile([RB, 1], f32)
            cte = consts.tile([1, RF], f32)
            te_sb = consts.tile([1, RF], f32)

            # ss: s chunk-pairs on partitions for the TensorE stationary
            # (pair dim outer -- dual-fp8 ldweights ISA rule); srow feeds
            # the broadcast. fp8 is exact for +-1. ss lands first so the
            # first fp8 matmul is gated only by its W tile.
            nc.vector.memset(ones[:], 1.0)
            nc.scalar.dma_start(out=ss[:], in_=s8[:, :])
            nc.scalar.dma_start(out=srow[:], in_=srow8[None, :])
            nc.scalar.dma_start(out=ct[:], in_=c_t[:, :])
            nc.scalar.dma_start(out=cte[:], in_=c_te[:, :])

            # Single sync HWDGE ring carries both W streams (1 bf16 tile
            # per 16 fp8 tiles); ps[b] accumulates TensorE dot products
            # across 64 chunk-pairs. The s broadcast (matmul + ACT copy
            # per 512-col chunk) rides the first 32 iterations of the PE
            # queue; DVE ops are gated by their W tile + the sb chunks
            # they read, all done well before the stream ends.
            ps = [psacc.tile([1, fb], f32, name=f"ps{b}") for b, fb in enumerate(BLOCKS)]
            NK2 = NKP // 2
            dve_slots = {3 + 4 * i: i for i in range(NCH)}  # k2 >= 4*cd+3: sb chunks written first
            for k2 in range(NK2):
                ring = nc.sync if k2 % 2 == 0 else nc.scalar
                tt = tpool.tile([P, 2, 2, RF], f8, name="ttile")
                if k2 == NK2 - 1:
                    nc.scalar.dma_start(out=tt[:, 0], in_=wte8[k2, :, 0 : 2 * RF])
                    nc.sync.dma_start(out=tt[:, 1], in_=wte8[k2, :, 2 * RF : 4 * RF])
                else:
                    ring.dma_start(out=tt[:], in_=wte8[k2, :, :])
                for ki in range(2):
                    k = 2 * k2 + ki
                    if k < N // BCF:
                        js = slice(k * BCF, (k + 1) * BCF)
                        pt = bcpsum.tile([P, BCF], f32)
                        nc.tensor.matmul(pt[:], ones[:], srow[:, js])
                        nc.scalar.copy(out=sb[:, js], in_=pt[:])
                    off = 0
                    for b, fb in enumerate(BLOCKS):
                        nc.tensor.matmul(
                            ps[b][:],
                            ss[:, :, k, None],
                            tt[:, ki, :, off : off + fb],
                            start=(k == 0),
                            stop=(k == NKP - 1),
                            perf_mode=mybir.MatmulPerfMode.DoubleRow,
                            skip_group_check=True,
                        )
                        off += fb
                if k2 in dve_slots:
                    cd = dve_slots[k2]
                    js = slice(cd * FD, (cd + 1) * FD)
                    wt = wpool.tile([RB, FD], bf, name="wtile")
                    (nc.scalar if k2 % 2 == 0 else nc.sync).dma_start(
                        out=wt[:], in_=w[:, js]
                    )
                    nc.vector.scalar_tensor_tensor(
                        out=dummy[:].broadcast_to([RB, FD]),
                        in0=wt[:],
                        scalar=1.0,
                        in1=sb[:RB, js],
                        op0=mybir.AluOpType.bypass,
                        op1=mybir.AluOpType.mult,
                        accum_out=partials[:, cd : cd + 1],
                    )

            # DVE-path epilogue (done mid-stream): act = sum(partials)+c.
            acc = consts.tile([RB, 1], f32)
            ob = consts.tile([RB, 1], f32)
            nc.vector.tensor_reduce(
                out=acc[:], in_=partials[:], axis=mybir.AxisListType.X,
                op=mybir.AluOpType.add,
            )
            nc.vector.tensor_tensor(
                out=ob[:], in0=acc[:], in1=ct[:], op=mybir.AluOpType.add,
            )
            nc.scalar.dma_start(out=out_o[:, :], in_=ob[:])

            # TensorE epilogue per block, right after its stop matmul;
            # each block's act slice ships as its own DMA so the adds and
            # output transfers pipeline.
            off = 0
            for b, fb in enumerate(BLOCKS):
                nc.vector.tensor_tensor(
                    out=te_sb[:, off : off + fb],
                    in0=ps[b][:],
                    in1=cte[:, off : off + fb],
                    op=mybir.AluOpType.add,
                )
                nc.scalar.dma_start(
                    out=out_te[:, off : off + fb], in_=te_sb[:, off : off + fb]
                )
                off += fb

    nc.finalize()
    return nc


def get_nc(RB_=None):
    if "nc" not in _CACHE:
        _CACHE["nc"] = _build_nc()
    return _CACHE["nc"]


def _plan(weights, state, bias):
    """Rows sorted by |c|: the lowest (sign-flip risk under fp8) fill the
    8*RB bf16 slots; the rest stream as fp8 on the TensorEngine."""
    diag = np.ascontiguousarray(np.diagonal(weights))
    corr = (N - 1) * bias - diag * state
    n_risky = int((np.abs(corr) < RISK_T).sum())
    assert n_risky <= RB * NCORES, f"too many risky rows: {n_risky}"
    order = np.argsort(np.abs(corr), kind="stable").astype(np.int64)
    return corr, order, RB


def make_in_maps(weights, state, bias, plan=None):
    weights = np.ascontiguousarray(weights, dtype=np.float32)
    state = np.ascontiguousarray(state, dtype=np.float32)
    bias = np.ascontiguousarray(bias, dtype=np.float32)
    corr, order, _ = plan if plan is not None else _plan(weights, state, bias)
    nb = RB * NCORES

    s8 = np.ascontiguousarray(
        state.reshape(NKP, 2, P).transpose(2, 1, 0).reshape(P, 2 * NKP)
    ).astype(ml_dtypes.float8_e4m3)
    srow8 = state.astype(ml_dtypes.float8_e4m3)

    in_maps = []
    for c in range(NCORES):
        brows = order[c * RB : (c + 1) * RB]
        frows = order[nb + c * RF : nb + (c + 1) * RF]
        w8f = weights[frows].astype(ml_dtypes.float8_e4m3)
        # W^T tiles merge two chunk-pairs for ~8 KiB DMA descriptor
        # lines: [k2][j_in 128][ki 2][pair 2][row] (pair dim outer)
        wte8 = np.ascontiguousarray(
            w8f.T.reshape(NKP // 2, 2, 2, P, RF).transpose(0, 3, 1, 2, 4)
        ).reshape(NKP // 2, P, 4 * RF)
        in_maps.append(
            {
                "wte8": wte8,
                "w": weights[brows].astype(ml_dtypes.bfloat16),
                "s8": s8,
                "srow8": srow8,
                "c_t": np.ascontiguousarray(corr[brows].reshape(RB, 1)),
                "c_te": np.ascontiguousarray(corr[frows].reshape(1, RF)),
            }
        )
    return in_maps


def gather(results, plan):
    corr, order, _ = plan
    nb = RB * NCORES
    act = np.empty(N, dtype=np.float32)
    for c, r in enumerate(results):
        act[order[c * RB : (c + 1) * RB]] = r["out_o"][:, 0]
        act[order[nb + c * RF : nb + (c + 1) * RF]] = r["out_te"][0]
    ns = np.where(act >= 0, np.float32(1.0), np.float32(-1.0))
    return act, ns


def kernel(weights, state, bias):
    from concourse.bass_utils import run_bass_kernel_spmd

    weights = np.ascontiguousarray(weights, dtype=np.float32)
    state = np.ascontiguousarray(state, dtype=np.float32)
    bias = np.ascontiguousarray(bias, dtype=np.float32)
    plan = _plan(weights, state, bias)
    nc = get_nc()
    in_maps = make_in_maps(weights, state, bias, plan)
    res = run_bass_kernel_spmd(nc, in_maps, list(range(NCORES)))
    return gather(res.results, plan)


# revision 6
# speedup vs baseline: 1.1057x; 1.1057x over previous
"""Hopfield neuron update kernel for 8 Trainium2 NeuronCores (fp8 stream).

act = W @ s - diag(W)*s + (N-1)*b;  new_state = sign(act)  (host-side).

Memory-bound on the W stream -> minimize streamed bytes: s is +-1 so
products are exact in any W precision; only the quantization of W
matters (act scale ~7e4 vs fp8-e4m3 row error <= ~16, 2e-4 relative).
Rows where |c| = |(N-1)b - diag*s| < 600 could flip new_state's sign
under fp8 error (|W@s| <= ~520 + err), so those (~485 of 16384) stream
in bf16 on the Vector-engine path; all other rows stream as 1-byte fp8
e4m3 (TRN FP8_EXP4-compatible, |W| < 6 << 240) through the
TensorEngine's Double-FP8 matmul (DoubleRow perf mode: K=256 per
instruction, 2 fp8 per lane-cycle; the k-pair dim of both operands must
be OUTER per the s3_lw_dual_fp8 ldweights ISA rule). ~34.6 MiB/core at
the ~22.4 GB/s x 16 DMA-engine per-NC HBM limit -> ~101 us stream, the
binding roofline; everything else hides under or drains right after it:
  - W^T tiles merge two chunk-pairs (7936 B per-partition lines --
    measured the best descriptor size: 3968 B lines ran 6% slower,
    16 KiB lines slower still) and alternate between the two HWDGE
    rings (sync/scalar) so each DMA engine round-robins two queues and
    per-queue stalls overlap (measured ~-10 us vs a single ring); the
    final tile splits in two so the PE drain after the last byte is
    short;
  - the rank-1 ones-outer-product broadcast of s (PE matmul + ACT copy
    per 512-col chunk) is interleaved into the first 32 chunk-pairs so
    the in-order PE queue reaches the first fp8 matmul immediately;
  - DVE bf16 dot products run mid-stream off 4 interleaved 512 KiB
    tiles scheduled in the first half of the ring (a DVE op must be
    emitted after the broadcast copies of every sb chunk it reads --
    earlier emission is a WAR race reading uninitialized SBUF);
  - per-PSUM-block epilogue add + output DMA pipeline behind the stop
    matmuls; corrections c fold host-side; new_state = sign(act) and
    the row un-permute are O(N) host postprocess.
Measured 117.3-117.5 us on the 8-core axon pod (baseline bf16: 211-221
us); the stream runs at ~95% of device HBM bandwidth.
"""

import os
import sys

import ml_dtypes
import numpy as np

for _p in ("/opt/trn_rl_repo", "/root/.axon_site/_ro/trn_rl_repo"):
    if os.path.isdir(_p) and _p not in sys.path:
        sys.path.insert(0, _p)

N = 16384
NCORES = 8
R = N // NCORES          # rows per core: 2048
P = 128                  # SBUF partitions
RISK_T = 600.0           # |c| below this -> bf16 row (margin ~373 on seed 0)
RB = 64                  # bf16 rows per core (>= ceil(risky/8), <= 128)
RF = R - RB              # TensorE fp8 rows per core: 1984
FD = 4096                # DVE op chunk (elems)
NCH = N // FD            # DVE chunks per row: 4
NKP = N // 256           # chunk-pairs for DoubleRow accumulation: 64
BLOCKS = (512, 512, 512, RF - 1536)  # PSUM out-free blocks
TBUFS = 8                # in-flight TensorE W^T tiles
WBUFS = 4                # in-flight DVE W tiles
BCF = 512                # broadcast chunk (PSUM bank max)

_CACHE = {}


def _build_nc():
    import concourse.bacc as bacc
    import concourse.mybir as mybir
    from concourse.tile import TileContext

    f32 = mybir.dt.float32
    bf = mybir.dt.bfloat16
    f8 = mybir.dt.float8e4
    nc = bacc.Bacc()

    wte8 = nc.dram_tensor("wte8", [NKP // 2, P, 4 * RF], f8, kind="ExternalInput")
    w = nc.dram_tensor("w", [RB, N], bf, kind="ExternalInput")
    s8 = nc.dram_tensor("s8", [P, 2 * NKP], f8, kind="ExternalInput")
    srow8 = nc.dram_tensor("srow8", [N], f8, kind="ExternalInput")
    c_t = nc.dram_tensor("c_t", [RB, 1], f32, kind="ExternalInput")
    c_te = nc.dram_tensor("c_te", [1, RF], f32, kind="ExternalInput")
    out_o = nc.dram_tensor("out_o", [RB, 1], f32, kind="ExternalOutput")
    out_te = nc.dram_tensor("out_te", [1, RF], f32, kind="ExternalOutput")

    with TileContext(nc) as tc:
        with (
            tc.tile_pool(name="consts", bufs=1) as consts,
            tc.tile_pool(name="tpool", bufs=TBUFS) as tpool,
            tc.tile_pool(name="wpool", bufs=WBUFS) as wpool,
            tc.tile_pool(name="psacc", bufs=1, space="PSUM") as psacc,
            tc.tile_pool(name="bcpsum", bufs=4, space="PSUM") as bcpsum,
        ):
            sb = consts.tile([P, N], f8)
            ss = consts.tile([P, 2, NKP], f8)
            srow = consts.tile([1, N], f8)
            ones = consts.tile([1, P], f8)
            dummy = consts.tile([RB, 1], f8)
            partials = consts.tile([RB, NCH], f32)
            ct = consts.tile([RB, 1], f32)
            cte = consts.tile([1, RF], f32)
            te_sb = consts.tile([1, RF], f32)

            # ss: s chunk-pairs on partitions for the TensorE stationary
            # (pair dim outer -- dual-fp8 ldweights ISA rule); srow feeds
            # the broadcast. fp8 is exact for +-1. ss lands first so the
            # first fp8 matmul is gated only by its W tile.
            nc.vector.memset(ones[:], 1.0)
            nc.scalar.dma_start(out=ss[:], in_=s8[:, :])
            nc.scalar.dma_start(out=srow[:], in_=srow8[None, :])
            nc.scalar.dma_start(out=ct[:], in_=c_t[:, :])
            nc.scalar.dma_start(out=cte[:], in_=c_te[:, :])

            # Single sync HWDGE ring carries both W streams (1 bf16 tile
            # per 16 fp8 tiles); ps[b] accumulates TensorE dot products
            # across 64 chunk-pairs. The s broadcast (matmul + ACT copy
            # per 512-col chunk) rides the first 32 iterations of the PE
            # queue; DVE ops are gated by their W tile + the sb chunks
            # they read, all done well before the stream ends.
            ps = [psacc.tile([1, fb], f32, name=f"ps{b}") for b, fb in enumerate(BLOCKS)]
            NK2 = NKP // 2
            dve_slots = {3 + 4 * i: i for i in range(NCH)}  # k2 >= 4*cd+3: sb chunks written first
            for k2 in range(NK2):
                ring = nc.sync if k2 % 2 == 0 else nc.scalar
                tt = tpool.tile([P, 2, 2, RF], f8, name="ttile")
                if k2 == NK2 - 1:
                    ring.dma_start(out=tt[:, 0], in_=wte8[k2, :, 0 : 2 * RF])
                    ring.dma_start(out=tt[:, 1], in_=wte8[k2, :, 2 * RF : 4 * RF])
                else:
                    ring.dma_start(out=tt[:], in_=wte8[k2, :, :])
                for ki in range(2):
                    k = 2 * k2 + ki
                    if k < N // BCF:
                        js = slice(k * BCF, (k + 1) * BCF)
                        pt = bcpsum.tile([P, BCF], f32)
                        nc.tensor.matmul(pt[:], ones[:], srow[:, js])
                        nc.scalar.copy(out=sb[:, js], in_=pt[:])
                    off = 0
                    for b, fb in enumerate(BLOCKS):
                        nc.tensor.matmul(
                            ps[b][:],
                            ss[:, :, k, None],
                            tt[:, ki, :, off : off + fb],
                            start=(k == 0),
                            stop=(k == NKP - 1),
                            perf_mode=mybir.MatmulPerfMode.DoubleRow,
                            skip_group_check=True,
                        )
                        off += fb
                if k2 in dve_slots:
                    cd = dve_slots[k2]
                    js = slice(cd * FD, (cd + 1) * FD)
                    wt = wpool.tile([RB, FD], bf, name="wtile")
                    nc.sync.dma_start(out=wt[:], in_=w[:, js])
                    nc.vector.scalar_tensor_tensor(
                        out=dummy[:].broadcast_to([RB, FD]),
                        in0=wt[:],
                        scalar=1.0,
                        in1=sb[:RB, js],
                        op0=mybir.AluOpType.bypass,
                        op1=mybir.AluOpType.mult,
                        accum_out=partials[:, cd : cd + 1],
                    )

            # DVE-path epilogue (done mid-stream): act = sum(partials)+c.
            acc = consts.tile([RB, 1], f32)
            ob = consts.tile([RB, 1], f32)
            nc.vector.tensor_reduce(
                out=acc[:], in_=partials[:], axis=mybir.AxisListType.X,
                op=mybir.AluOpType.add,
            )
            nc.vector.tensor_tensor(
                out=ob[:], in0=acc[:], in1=ct[:], op=mybir.AluOpType.add,
            )
            nc.scalar.dma_start(out=out_o[:, :], in_=ob[:])

            # TensorE epilogue per block, right after its stop matmul;
            # each block's act slice ships as its own DMA so the adds and
            # output transfers pipeline.
            off = 0
            for b, fb in enumerate(BLOCKS):
                nc.vector.tensor_tensor(
                    out=te_sb[:, off : off + fb],
                    in0=ps[b][:],
                    in1=cte[:, off : off + fb],
                    op=mybir.AluOpType.add,
                )
                nc.scalar.dma_start(
                    out=out_te[:, off : off + fb], in_=te_sb[:, off : off + fb]
                )
                off += fb

    nc.finalize()
    return nc


def get_nc(RB_=None):
    if "nc" not in _CACHE:
        _CACHE["nc"] = _build_nc()
    return _CACHE["nc"]


def _plan(weights, state, bias):
    """Rows sorted by |c|: the lowest (sign-flip risk under fp8) fill the
    8*RB bf16 slots; the rest stream as fp8 on the TensorEngine."""
    diag = np.ascontiguousarray(np.diagonal(weights))
    corr = (N - 1) * bias - diag * state
    n_risky = int((np.abs(corr) < RISK_T).sum())
    assert n_risky <= RB * NCORES, f"too many risky rows: {n_risky}"
    order = np.argsort(np.abs(corr), kind="stable").astype(np.int64)
    return corr, order, RB


def make_in_maps(weights, state, bias, plan=None):
    weights = np.ascontiguousarray(weights, dtype=np.float32)
    state = np.ascontiguousarray(state, dtype=np.float32)
    bias = np.ascontiguousarray(bias, dtype=np.float32)
    corr, order, _ = plan if plan is not None else _plan(weights, state, bias)
    nb = RB * NCORES

    s8 = np.ascontiguousarray(
        state.reshape(NKP, 2, P).transpose(2, 1, 0).reshape(P, 2 * NKP)
    ).astype(ml_dtypes.float8_e4m3)
    srow8 = state.astype(ml_dtypes.float8_e4m3)

    in_maps = []
    for c in range(NCORES):
        brows = order[c * RB : (c + 1) * RB]
        frows = order[nb + c * RF : nb + (c + 1) * RF]
        w8f = weights[frows].astype(ml_dtypes.float8_e4m3)
        # W^T tiles merge two chunk-pairs for ~8 KiB DMA descriptor
        # lines: [k2][j_in 128][ki 2][pair 2][row] (pair dim outer)
        wte8 = np.ascontiguousarray(
            w8f.T.reshape(NKP // 2, 2, 2, P, RF).transpose(0, 3, 1, 2, 4)
        ).reshape(NKP // 2, P, 4 * RF)
        in_maps.append(
            {
                "wte8": wte8,
                "w": weights[brows].astype(ml_dtypes.bfloat16),
                "s8": s8,
                "srow8": srow8,
                "c_t": np.ascontiguousarray(corr[brows].reshape(RB, 1)),
                "c_te": np.ascontiguousarray(corr[frows].reshape(1, RF)),
            }
        )
    return in_maps


def gather(results, plan):
    corr, order, _ = plan
    nb = RB * NCORES
    act = np.empty(N, dtype=np.float32)
    for c, r in enumerate(results):
        act[order[c * RB : (c + 1) * RB]] = r["out_o"][:, 0]
        act[order[nb + c * RF : nb + (c + 1) * RF]] = r["out_te"][0]
    ns = np.where(act >= 0, np.float32(1.0), np.float32(-1.0))
    return act, ns


def kernel(weights, state, bias):
    from concourse.bass_utils import run_bass_kernel_spmd

    weights = np.ascontiguousarray(weights, dtype=np.float32)
    state = np.ascontiguousarray(state, dtype=np.float32)
    bias = np.ascontiguousarray(bias, dtype=np.float32)
    plan = _plan(weights, state, bias)
    nc = get_nc()
    in_maps = make_in_maps(weights, state, bias, plan)
    res = run_bass_kernel_spmd(nc, in_maps, list(range(NCORES)))
    return gather(res.results, plan)
